# revision 1
# baseline (speedup 1.0000x reference)
"""Trainium2 Bass kernel for the arm-sampling rollout problem.

Math: the reference's 2048-step scan x <- x - (A@x)*dt with
A = P diag(exp(D)) P^-1 has the closed form
    hidden[k] = P diag(lam_i^k) P^-1 x0,   lam_i = 1 - dt*exp(D_i)
so actions^T[ch, k] = tanh(sum_i G[ch,i] * c_i * lam_i^k + bm[ch]) with
G = Wm @ P and c = P^-1 x0. c is obtained on-device: unpivoted
Gauss-Jordan on [P^T | I] (same pivot sequence as P; well-conditioned
for this problem family) gives Q = P^-T, then c = matmul(lhsT=Q, rhs=x0).
The output is the memory-bound broadcast
    out[arm, j] = 150*eps[arm, j] + 15000*act_flat[j]
over a [5000, 4096] array, sharded 625 arms per core across 8 cores.

DMA model (measured): each dma_start is serviced at ~27GB/s by a single
DMA engine; aggregate bandwidth (~400GB/s) requires many concurrent
transfers. So the bulk stream is one [125, 20480] supertile (5 arms per
80KB-contiguous row) moved as 16 row-group transfers per direction,
while every prologue load is a small gpsimd (SWDGE) transfer so the
sync/scalar HWDGE queues and the compute engines stay free for the
broadcast-matrix critical path.
"""

import numpy as np

import concourse.bass as bass
import concourse.bacc as bacc
import concourse.mybir as mybir
import concourse.tile as tile
from concourse.bass_utils import run_bass_kernel_spmd

N_ARMS = 5000
N_STEPS = 2048
H = 10
F = 2 * N_STEPS  # 4096 flattened per-arm elements
N_CORES = 8
ARMS_PER_CORE = N_ARMS // N_CORES  # 625
APR = 5  # arms per supertile row
ROWS = ARMS_PER_CORE // APR  # 125
WIDE = APR * F  # 20480
FP = mybir.dt.float32

_NC_CACHE: dict = {}


def build_nc():
    AFT = mybir.ActivationFunctionType
    ALU = mybir.AluOpType

    nc = bacc.Bacc(
        "TRN2",
        target_bir_lowering=False,
        debug=False,
        enable_asserts=True,
        num_devices=N_CORES,
    )

    eps_d = nc.dram_tensor("eps", [ARMS_PER_CORE, F], FP, kind="ExternalInput")
    tgt_d = nc.dram_tensor("target", [2], FP, kind="ExternalInput")
    D_d = nc.dram_tensor("D", [H], FP, kind="ExternalInput")
    P_d = nc.dram_tensor("P", [H, H], FP, kind="ExternalInput")
    W1_d = nc.dram_tensor("W1", [256, 2], FP, kind="ExternalInput")
    b1_d = nc.dram_tensor("b1", [256], FP, kind="ExternalInput")
    W2_d = nc.dram_tensor("W2", [H, 256], FP, kind="ExternalInput")
    b2_d = nc.dram_tensor("b2", [H], FP, kind="ExternalInput")
    Wm_d = nc.dram_tensor("Wm", [2, H], FP, kind="ExternalInput")
    bm_d = nc.dram_tensor("bm", [2], FP, kind="ExternalInput")
    out_d = nc.dram_tensor("out", [ARMS_PER_CORE, F], FP, kind="ExternalOutput")

    GROUPS = []
    g = 0
    while g < ROWS:
        g2 = min(g + 8, ROWS)
        GROUPS.append((g, g2))
        g = g2

    with tile.TileContext(nc) as tc:
        with (
            tc.tile_pool(name="sbc", bufs=1) as sbc,
            tc.tile_pool(name="sbgj", bufs=2) as sbgj,
            tc.tile_pool(name="sbeps", bufs=1) as sbeps,
            tc.tile_pool(name="psa", bufs=2, space=bass.MemorySpace.PSUM) as psa,
            tc.tile_pool(name="psbc", bufs=2, space=bass.MemorySpace.PSUM) as psbc,
            tc.tile_pool(name="psact", bufs=2, space=bass.MemorySpace.PSUM) as psact,
            tc.tile_pool(name="psB", bufs=2, space=bass.MemorySpace.PSUM) as psB,
        ):
            # ---------- small loads (gpsimd/SWDGE), GJ-critical first --------
            pT = sbc.tile([H, H], FP, tag="pT")
            nc.gpsimd.dma_start(pT[:], P_d.ap().rearrange("m k -> k m"))
            ds = sbc.tile([H, 1], FP, tag="ds")
            nc.gpsimd.dma_start(ds[:], D_d.ap()[:, None])
            w1n0 = sbc.tile([128, 2], FP, tag="w1n0")
            nc.gpsimd.dma_start(w1n0[:], W1_d.ap()[0:128, :])
            w1n1 = sbc.tile([128, 2], FP, tag="w1n1")
            nc.gpsimd.dma_start(w1n1[:], W1_d.ap()[128:256, :])
            b1n = sbc.tile([1, 256], FP, tag="b1n")
            nc.gpsimd.dma_start(b1n[:], b1_d.ap()[None, :])
            tgtr = sbc.tile([1, 2], FP, tag="tgtr")
            nc.gpsimd.dma_start(tgtr[:], tgt_d.ap()[None, :])
            w2n = sbc.tile([H, 256], FP, tag="w2n")
            nc.gpsimd.dma_start(w2n[:], W2_d.ap())
            p_sb = sbc.tile([H, H], FP, tag="p_sb")
            nc.gpsimd.dma_start(p_sb[:], P_d.ap())
            wmT = sbc.tile([H, 2], FP, tag="wmT")
            nc.gpsimd.dma_start(wmT[:], Wm_d.ap().rearrange("m k -> k m"))
            b2s = sbc.tile([H, 1], FP, tag="b2s")
            nc.gpsimd.dma_start(b2s[:], b2_d.ap()[:, None])
            bm0 = sbc.tile([1, 1], FP, tag="bm0")
            nc.gpsimd.dma_start(bm0[:], bm_d.ap()[0:1][:, None])
            bm1 = sbc.tile([1, 1], FP, tag="bm1")
            nc.gpsimd.dma_start(bm1[:], bm_d.ap()[1:2][:, None])

            # ---------- bulk in: 5 whole-tile transfers on sync --------------
            eps_tiles = []
            for r in range(0, ARMS_PER_CORE, 128):
                pt = min(128, ARMS_PER_CORE - r)
                t = sbeps.tile([128, F], FP, tag="eps" + str(r))
                nc.sync.dma_start(t[0:pt, :], eps_d.ap()[r : r + pt, :])
                eps_tiles.append((t, r, pt))

            ones = sbc.tile([1, 128], FP, tag="ones")
            nc.vector.memset(ones[:], 1.0)

            # idm[p, j] = 1 if p == j (via iota p-j then ==0)
            idi = sbc.tile([H, H], mybir.dt.int32, tag="idi")
            nc.gpsimd.iota(idi[:], pattern=[[-1, H]], base=0, channel_multiplier=1)
            idm = sbc.tile([H, H], FP, tag="idm")
            nc.vector.tensor_scalar(idm[:], idi[:], 0, None, ALU.is_equal)
            # oht[:, 10k:10k+10] = matrix with row k all-ones (lhsT that
            # broadcasts row k of the GJ tableau to every partition).
            oht = sbc.tile([H, H * H], FP, tag="oht")
            oht3 = oht[:].rearrange("p (k r) -> p k r", r=H)
            for r in range(H):
                nc.vector.tensor_copy(oht3[:, :, r : r + 1], idm[:, :, None])

            # ---------- lam = 1 - 0.01*exp(D); lnlam; V = lam^k --------------
            es = sbc.tile([H, 1], FP, tag="es")
            nc.scalar.activation(es[:], ds[:], AFT.Exp)
            lam = sbc.tile([H, 1], FP, tag="lam")
            nc.vector.tensor_scalar(lam[:], es[:], -0.01, 1.0, ALU.mult, ALU.add)
            lnl = sbc.tile([H, 1], FP, tag="lnl")
            nc.scalar.activation(lnl[:], lam[:], AFT.Ln)
            ki = sbc.tile([H, N_STEPS], mybir.dt.int32, tag="ki")
            nc.gpsimd.iota(ki[:], pattern=[[1, N_STEPS]], base=0, channel_multiplier=0)
            kf = sbc.tile([H, N_STEPS], FP, tag="kf")
            nc.vector.tensor_copy(kf[:], ki[:])
            vc = sbc.tile([H, N_STEPS], FP, tag="vc")
            nc.scalar.activation(vc[:], kf[:], AFT.Exp, scale=lnl[:])

            # ---------- Gauss-Jordan on [P^T | I] -> Q = P^-T ----------------
            aug = sbgj.tile([H, 2 * H], FP, tag="aug")
            nc.vector.tensor_copy(aug[:, 0:H], pT[:])
            nc.vector.tensor_copy(aug[:, H : 2 * H], idm[:])
            for k in range(H):
                bc = psbc.tile([H, 2 * H], FP, tag="bc")
                nc.tensor.matmul(bc[:], oht[:, H * k : H * k + H], aug[:])
                piv = sbgj.tile([H, 1], FP, tag="piv")
                nc.vector.reciprocal(piv[:], bc[:, k : k + 1])
                S = sbgj.tile([H, 2 * H], FP, tag="S")
                nc.vector.tensor_scalar_mul(S[:], bc[:], piv[:])
                fn = sbgj.tile([H, 1], FP, tag="fn")
                nc.vector.tensor_sub(fn[:], idm[:, k : k + 1], aug[:, k : k + 1])
                aug2 = sbgj.tile([H, 2 * H], FP, tag="aug")
                nc.vector.scalar_tensor_tensor(
                    aug2[:], S[:], fn[:], aug[:], ALU.mult, ALU.add
                )
                aug = aug2

            # ---------- x0 = W2 @ relu(W1 @ target + b1) + b2 ----------------
            tbp = psa.tile([128, 2], FP, tag="mm")
            nc.tensor.matmul(tbp[:], ones[:], tgtr[:])
            tb = sbc.tile([128, 2], FP, tag="tb")
            nc.vector.tensor_copy(tb[:], tbp[:])
            b1p0 = psa.tile([128, 1], FP, tag="mm")
            nc.tensor.matmul(
                b1p0[:], b1n[0:1, 0:128], ones[0:1, 0:1], is_transpose=True
            )
            b1a = sbc.tile([128, 1], FP, tag="b1a")
            nc.vector.tensor_copy(b1a[:], b1p0[:])
            b1p1 = psa.tile([128, 1], FP, tag="mm")
            nc.tensor.matmul(
                b1p1[:], b1n[0:1, 128:256], ones[0:1, 0:1], is_transpose=True
            )
            b1b = sbc.tile([128, 1], FP, tag="b1b")
            nc.vector.tensor_copy(b1b[:], b1p1[:])
            w2tp0 = psa.tile([128, H], FP, tag="mm")
            nc.tensor.matmul(w2tp0[:], w2n[:, 0:128], idm[:], is_transpose=True)
            w2t0 = sbc.tile([128, H], FP, tag="w2t0")
            nc.vector.tensor_copy(w2t0[:], w2tp0[:])
            w2tp1 = psa.tile([128, H], FP, tag="mm")
            nc.tensor.matmul(w2tp1[:], w2n[:, 128:256], idm[:], is_transpose=True)
            w2t1 = sbc.tile([128, H], FP, tag="w2t1")
            nc.vector.tensor_copy(w2t1[:], w2tp1[:])

            u0 = sbc.tile([128, 1], FP, tag="u0")
            nc.vector.tensor_scalar_mul(u0[:], w1n0[:, 1:2], tb[:, 1:2])
            hp0 = sbc.tile([128, 1], FP, tag="hp0")
            nc.vector.scalar_tensor_tensor(
                hp0[:], w1n0[:, 0:1], tb[:, 0:1], u0[:], ALU.mult, ALU.add
            )
            h0 = sbc.tile([128, 1], FP, tag="h0")
            nc.scalar.activation(h0[:], hp0[:], AFT.Relu, bias=b1a[:], scale=1.0)
            u1 = sbc.tile([128, 1], FP, tag="u1")
            nc.vector.tensor_scalar_mul(u1[:], w1n1[:, 1:2], tb[:, 1:2])
            hp1 = sbc.tile([128, 1], FP, tag="hp1")
            nc.vector.scalar_tensor_tensor(
                hp1[:], w1n1[:, 0:1], tb[:, 0:1], u1[:], ALU.mult, ALU.add
            )
            h1 = sbc.tile([128, 1], FP, tag="h1")
            nc.scalar.activation(h1[:], hp1[:], AFT.Relu, bias=b1b[:], scale=1.0)
            x0p = psa.tile([H, 1], FP, tag="mm")
            nc.tensor.matmul(x0p[:], w2t0[:], h0[:], start=True, stop=False)
            nc.tensor.matmul(x0p[:], w2t1[:], h1[:], start=False, stop=True)
            x0s = sbc.tile([H, 1], FP, tag="x0s")
            nc.scalar.activation(x0s[:], x0p[:], AFT.Identity, bias=b2s[:], scale=1.0)

            # ---------- c[m] = sum_k Q[k, m] x0[k]  (Q = P^-T) ---------------
            cp = psa.tile([H, 1], FP, tag="mm")
            nc.tensor.matmul(cp[:], aug[:, H : 2 * H], x0s[:])

            # ---------- G^T = (Wm @ P)^T; fold c in: gts = G^T * c -----------
            gtp = psa.tile([H, 2], FP, tag="mm")
            nc.tensor.matmul(gtp[:], p_sb[:], wmT[:])
            gts = sbc.tile([H, 2], FP, tag="gts")
            nc.vector.tensor_scalar_mul(gts[:], gtp[:], cp[:, 0:1])

            # ---------- actions: per-channel rows on partition 0 -------------
            ats = sbc.tile([1, F], FP, tag="ats")
            for ch in range(2):
                bmt = bm0 if ch == 0 else bm1
                for j in range(N_STEPS // 512):
                    atp = psact.tile([1, 512], FP, tag="actT")
                    nc.tensor.matmul(
                        atp[:], gts[:, ch : ch + 1], vc[:, 512 * j : 512 * (j + 1)]
                    )
                    nc.scalar.activation(
                        ats[:, ch * N_STEPS + 512 * j : ch * N_STEPS + 512 * (j + 1)],
                        atp[:],
                        AFT.Tanh,
                        bias=bmt[:],
                        scale=1.0,
                    )

            # ---------- B[p, 2t+ch] = 15000 * ats[ch, t] on 128 partitions ---
            Bsb = sbc.tile([128, F], FP, tag="B")
            B3 = Bsb[:].rearrange("p (t m) -> p t m", m=2)
            for ch in range(2):
                for j in range(N_STEPS // 512):
                    bp = psB.tile([128, 512], FP, tag="B")
                    nc.tensor.matmul(
                        bp[:],
                        ones[:],
                        ats[:, ch * N_STEPS + 512 * j : ch * N_STEPS + 512 * (j + 1)],
                    )
                    nc.scalar.activation(
                        B3[:, 512 * j : 512 * (j + 1), ch : ch + 1],
                        bp[:, :, None],
                        AFT.Copy,
                        scale=15000.0,
                    )

            # ---------- main: out = 150*eps + B, per tile; outs on 2 queues --
            for i, (t, r, pt) in enumerate(eps_tiles):
                nc.vector.scalar_tensor_tensor(
                    t[0:pt, :], t[0:pt, :], 150.0, Bsb[0:pt, :], ALU.mult, ALU.add
                )
                eng = nc.sync if i % 2 == 0 else nc.scalar
                eng.dma_start(out_d.ap()[r : r + pt, :], t[0:pt, :])

    nc.compile()
    return nc


def get_nc():
    if "nc" not in _NC_CACHE:
        _NC_CACHE["nc"] = build_nc()
    return _NC_CACHE["nc"]


def kernel(**inputs):
    nc = get_nc()
    eps = np.ascontiguousarray(
        np.asarray(inputs["eps"], dtype=np.float32).reshape(N_ARMS, F)
    )
    small = {
        k: np.ascontiguousarray(np.asarray(inputs[k], dtype=np.float32))
        for k in ["target", "D", "P", "W1", "b1", "W2", "b2", "Wm", "bm"]
    }
    in_maps = [
        {**small, "eps": eps[i * ARMS_PER_CORE : (i + 1) * ARMS_PER_CORE]}
        for i in range(N_CORES)
    ]
    res = run_bass_kernel_spmd(nc, in_maps, core_ids=list(range(N_CORES)))
    out = np.concatenate([res.results[i]["out"] for i in range(N_CORES)], axis=0)
    return out.reshape(N_ARMS, 2, N_STEPS)



# revision 5
# speedup vs baseline: 1.3041x; 1.3041x over previous
"""Trainium2 Bass kernel for the arm-sampling rollout problem.

Math: the reference's 2048-step scan x <- x - (A@x)*dt with
A = P diag(exp(D)) P^-1 has the closed form
    hidden[k] = P diag(lam_i^k) P^-1 x0,   lam_i = 1 - dt*exp(D_i)
so actions^T[ch, k] = tanh(sum_i G[ch,i] * c_i * lam_i^k + bm[ch]) with
G = Wm @ P and c = P^-1 x0 (via on-device Gauss-Jordan on [P^T | I]).
The output is the memory-bound broadcast
    out[arm, j] = 150*eps[arm, j] + 15000*act_flat[j]
over a [5000, 4096] array, 625 arms per core across 8 cores.

DMA model (measured from NTFF): a dma_start's descriptors (16KB per
partition-row) are dealt across the 16 DMA engines (26.8GB/s each,
~430GB/s aggregate) only while other transfers co-reside in the
queue's dispatch window; a transfer left alone in the window is
serviced by ONE engine at 26.8GB/s. So the bulk stream is many ~1MB
transfers on one HWDGE queue, kept continuously populated, with small
dummy transfers appended so no real transfer ends up alone.

Prologue latency: the 9 small parameter tensors are host-packed into
two partition-layout arrays (pk3 [3,258], pk10 [10,280]) so each loads
as a handful of descriptors in ~1us, and the b1/target/bm constants
ride inside the same matmuls (bias rows / rhs columns) instead of
needing separate layouts. gpsimd only runs iotas; the tanh action row
is built 2-channels-at-a-time ([2,512] matmuls), broadcast 128-wide
with bf16 ones@row matmuls (tanh in [-1,1], bf16 quantization adds
<=2^-9 relative on the 15000-scaled term), and scaled into Bsb by
scalar/vector copies in parallel.
"""

import numpy as np

import concourse.bass as bass
import concourse.bacc as bacc
import concourse.mybir as mybir
import concourse.tile as tile
from concourse.bass_utils import run_bass_kernel_spmd

N_ARMS = 5000
N_STEPS = 2048
H = 10
F = 2 * N_STEPS  # 4096 flattened per-arm elements
N_CORES = 8
ARMS_PER_CORE = N_ARMS // N_CORES  # 625
FP = mybir.dt.float32
BF = mybir.dt.bfloat16

_NC_CACHE: dict = {}


def build_nc():
    AFT = mybir.ActivationFunctionType
    ALU = mybir.AluOpType

    nc = bacc.Bacc(
        "TRN2",
        target_bir_lowering=False,
        debug=False,
        enable_asserts=True,
        num_devices=N_CORES,
    )

    eps_d = nc.dram_tensor("eps", [ARMS_PER_CORE, F], FP, kind="ExternalInput")
    pk3_d = nc.dram_tensor("pk3", [3, 258], FP, kind="ExternalInput")
    pk10_d = nc.dram_tensor("pk10", [10, 280], FP, kind="ExternalInput")
    out_d = nc.dram_tensor("out", [ARMS_PER_CORE, F], FP, kind="ExternalOutput")
    dscr_d = nc.dram_tensor("dscr", [10, 280], FP, kind="Internal")

    # eps row-groups: 9 x 64 + 1 x 49 rows; SBUF tiles of 128 rows (last 113)
    TILE_ROWS = [(0, 128), (128, 256), (256, 384), (384, 512), (512, 625)]
    IN_GROUPS = [(r, min(r + 64, ARMS_PER_CORE)) for r in range(0, ARMS_PER_CORE, 64)]

    with tile.TileContext(nc) as tc:
        with (
            tc.tile_pool(name="sbc", bufs=1) as sbc,
            tc.tile_pool(name="sbgj", bufs=2) as sbgj,
            tc.tile_pool(name="sbeps", bufs=1) as sbeps,
            tc.tile_pool(name="psa", bufs=3, space=bass.MemorySpace.PSUM) as psa,
            tc.tile_pool(name="psbc", bufs=2, space=bass.MemorySpace.PSUM) as psbc,
            tc.tile_pool(name="psB", bufs=2, space=bass.MemorySpace.PSUM) as psB,
        ):
            # ---------- gpsimd: iotas only (no input deps) --------------------
            idi = sbc.tile([H, H], mybir.dt.int32, tag="idi")
            nc.gpsimd.iota(idi[:], pattern=[[-1, H]], base=0, channel_multiplier=1)
            ki = sbc.tile([H, N_STEPS], mybir.dt.int32, tag="ki")
            nc.gpsimd.iota(ki[:], pattern=[[1, N_STEPS]], base=0, channel_multiplier=0)

            # ---------- vector: constants built pre-input ---------------------
            ones_bf = sbc.tile([1, 128], BF, tag="ones_bf")
            nc.vector.memset(ones_bf[:], 1.0)
            idm = sbc.tile([H, H], FP, tag="idm")
            nc.vector.tensor_scalar(idm[:], idi[:], 0, None, ALU.is_equal)
            # oht[:, 10k:10k+10]: lhsT that broadcasts row k to all partitions
            oht = sbc.tile([H, H * H], FP, tag="oht")
            oht3 = oht[:].rearrange("p (k r) -> p k r", r=H)
            for r in range(H):
                nc.vector.tensor_copy(oht3[:, :, r : r + 1], idm[:, :, None])
            kf = sbc.tile([H, N_STEPS], FP, tag="kf")
            nc.vector.tensor_copy(kf[:], ki[:])

            # ---------- sync HWDGE: packed smalls, then bulk eps --------------
            pk3 = sbc.tile([3, 258], FP, tag="pk3")
            nc.sync.dma_start(pk3[:], pk3_d.ap())
            eps_tiles = []
            for r0, r1 in TILE_ROWS:
                t = sbeps.tile([128, F], FP, tag="eps" + str(r0))
                eps_tiles.append((t, r0, r1 - r0))
            for g0, g1 in IN_GROUPS:
                ti = g0 // 128
                t, r0, _ = eps_tiles[ti]
                nc.sync.dma_start(t[g0 - r0 : g1 - r0, :], eps_d.ap()[g0:g1, :])
            # dummy input so the last real group is never alone in the window
            dscr_in = sbc.tile([4, F], FP, tag="dscr_in")
            nc.sync.dma_start(dscr_in[:], eps_d.ap()[0:4, :])

            # ---------- scalar HWDGE: pk10 + its dummy companion --------------
            pk10 = sbc.tile([10, 280], FP, tag="pk10")
            nc.scalar.dma_start(pk10[:], pk10_d.ap())
            nc.scalar.dma_start(dscr_d.ap(), pk10[:])

            # pk10 layout: [:,0:10]=P^T  [:,10:20]=P  [:,20:22]=Wm^T
            #              [:,22]=D  [:,23]=b2  [:,24:280]=W2
            pT = pk10[:, 0:10]
            p_nat = pk10[:, 10:20]
            wmT = pk10[:, 20:22]
            dcol = pk10[:, 22:23]
            b2col = pk10[:, 23:24]
            w2 = pk10[:, 24:280]
            # pk3 layout: rows0-1=W1^T, row2=b1; col256=[t0,t1,1]; col257=[bm0,bm1,0]
            tgt1 = pk3[:, 256:257]
            bm2 = pk3[0:2, 257:258]

            # ---------- lam = 1 - 0.01*exp(D); vc = lam^k ---------------------
            es = sbc.tile([H, 1], FP, tag="es")
            nc.scalar.activation(es[:], dcol, AFT.Exp)
            lam = sbc.tile([H, 1], FP, tag="lam")
            nc.vector.tensor_scalar(lam[:], es[:], -0.01, 1.0, ALU.mult, ALU.add)
            lnl = sbc.tile([H, 1], FP, tag="lnl")
            nc.scalar.activation(lnl[:], lam[:], AFT.Ln)
            vc = sbc.tile([H, N_STEPS], FP, tag="vc")
            nc.scalar.activation(vc[:], kf[:], AFT.Exp, scale=lnl[:])

            # ---------- h = relu(W1 t + b1) via augmented-contraction mm ------
            hp0 = psa.tile([128, 1], FP, tag="mm")
            nc.tensor.matmul(hp0[:], pk3[:, 0:128], tgt1)
            hp1 = psa.tile([128, 1], FP, tag="mm")
            nc.tensor.matmul(hp1[:], pk3[:, 128:256], tgt1)
            h0 = sbc.tile([128, 1], FP, tag="h0")
            nc.scalar.activation(h0[:], hp0[:], AFT.Relu)
            h1 = sbc.tile([128, 1], FP, tag="h1")
            nc.scalar.activation(h1[:], hp1[:], AFT.Relu)

            # ---------- W2^T halves via PE transpose --------------------------
            w2tp0 = psa.tile([128, H], FP, tag="mm")
            nc.tensor.matmul(w2tp0[:], w2[:, 0:128], idm[:], is_transpose=True)
            w2tp1 = psa.tile([128, H], FP, tag="mm")
            nc.tensor.matmul(w2tp1[:], w2[:, 128:256], idm[:], is_transpose=True)
            w2t0 = sbc.tile([128, H], FP, tag="w2t0")
            nc.vector.tensor_copy(w2t0[:], w2tp0[:])
            w2t1 = sbc.tile([128, H], FP, tag="w2t1")
            nc.vector.tensor_copy(w2t1[:], w2tp1[:])

            # ---------- G^T = P^T Wm^T --------------------------------------
            gtp = psa.tile([H, 2], FP, tag="mm")
            nc.tensor.matmul(gtp[:], p_nat, wmT)

            # ---------- Gauss-Jordan on [P^T | I] -> Q = P^-T -----------------
            aug = sbgj.tile([H, 2 * H], FP, tag="aug")
            nc.vector.tensor_copy(aug[:, 0:H], pT)
            nc.vector.tensor_copy(aug[:, H : 2 * H], idm[:])
            for k in range(H):
                # fn_k = e_k - aug[:,k] on gpsimd, parallel with the PE matmul
                fn = sbgj.tile([H, 1], FP, tag="fn")
                nc.gpsimd.tensor_sub(fn[:], idm[:, k : k + 1], aug[:, k : k + 1])
                bc = psbc.tile([H, 2 * H], FP, tag="bc")
                nc.tensor.matmul(bc[:], oht[:, H * k : H * k + H], aug[:])
                piv = sbgj.tile([H, 1], FP, tag="piv")
                nc.vector.reciprocal(piv[:], bc[:, k : k + 1])
                S = sbgj.tile([H, 2 * H], FP, tag="S")
                nc.vector.tensor_scalar_mul(S[:], bc[:], piv[:])
                aug2 = sbgj.tile([H, 2 * H], FP, tag="aug")
                nc.vector.scalar_tensor_tensor(
                    aug2[:], S[:], fn[:], aug[:], ALU.mult, ALU.add
                )
                aug = aug2

            # ---------- x0 = W2 h + b2; c = Q x0; gts = G^T * c ---------------
            x0p = psa.tile([H, 1], FP, tag="mm")
            nc.tensor.matmul(x0p[:], w2t0[:], h0[:], start=True, stop=False)
            nc.tensor.matmul(x0p[:], w2t1[:], h1[:], start=False, stop=True)
            x0s = sbc.tile([H, 1], FP, tag="x0s")
            nc.scalar.activation(x0s[:], x0p[:], AFT.Identity, bias=b2col, scale=1.0)
            cp = psa.tile([H, 1], FP, tag="mm")
            nc.tensor.matmul(cp[:], aug[:, H : 2 * H], x0s[:])
            gts = sbc.tile([H, 2], FP, tag="gts")
            nc.vector.tensor_scalar_mul(gts[:], gtp[:], cp[:, 0:1])

            # ---------- actions (both channels per mm), tanh -> bf16 ----------
            ats2 = sbc.tile([2, N_STEPS], BF, tag="ats2")
            # ch1's row hopped to partition 0 (matmul rhs needs base 0/32/64)
            ats1 = sbc.tile([1, N_STEPS], BF, tag="ats1")
            NCH = N_STEPS // 512  # 4 chunks of 512 steps
            pre_tiles = []
            for j in range(NCH):
                pre = psa.tile([2, 512], FP, tag="mm")
                nc.tensor.matmul(pre[:], gts[:], vc[:, 512 * j : 512 * (j + 1)])
                pre_tiles.append(pre)
            for j in range(NCH):
                nc.scalar.activation(
                    ats2[:, 512 * j : 512 * (j + 1)],
                    pre_tiles[j][:],
                    AFT.Tanh,
                    bias=bm2,
                    scale=1.0,
                )
                if j % 2 == 1:
                    c0, c1 = 512 * (j - 1), 512 * (j + 1)
                    nc.scalar.dma_start(ats1[0:1, c0:c1], ats2[1:2, c0:c1])

            # ---------- B[p, 2t+ch] = 15000*tanh(...) broadcast 128-wide ------
            Bsb = sbc.tile([128, F], FP, tag="B")
            B3 = Bsb[:].rearrange("p (t m) -> p t m", m=2)
            for j in range(NCH):
                for ch in range(2):
                    src = ats2 if ch == 0 else ats1
                    bp = psB.tile([128, 512], FP, tag="B")
                    nc.tensor.matmul(
                        bp[:], ones_bf[:], src[0:1, 512 * j : 512 * (j + 1)]
                    )
                    dst = B3[:, 512 * j : 512 * (j + 1), ch : ch + 1]
                    if ch == 0:
                        nc.scalar.activation(
                            dst, bp[:, :, None], AFT.Copy, scale=15000.0
                        )
                    else:
                        nc.vector.tensor_scalar(
                            dst, bp[:, :, None], 15000.0, None, ALU.mult
                        )

            # ---------- main: out = 150*eps + B, half-tiles; outs on sync -----
            HW = F // 2  # 2048 col half
            for t, r0, pt in eps_tiles:
                for hh in range(2):
                    c0, c1 = hh * HW, (hh + 1) * HW
                    nc.vector.scalar_tensor_tensor(
                        t[0:pt, c0:c1],
                        t[0:pt, c0:c1],
                        150.0,
                        Bsb[0:pt, c0:c1],
                        ALU.mult,
                        ALU.add,
                    )
                    if pt == 128:
                        nc.sync.dma_start(
                            out_d.ap()[r0 : r0 + pt, c0:c1], t[0:pt, c0:c1]
                        )
                    else:
                        # last tile: taper into col-quarters to shrink the tail
                        qw = HW // 2
                        for q in range(2):
                            qc0 = c0 + q * qw
                            nc.sync.dma_start(
                                out_d.ap()[r0 : r0 + pt, qc0 : qc0 + qw],
                                t[0:pt, qc0 : qc0 + qw],
                            )
            # dummy output companion for the final window
            nc.sync.dma_start(dscr_d.ap()[0:4, :], pk10[0:4, :])

    nc.compile()
    return nc


def get_nc():
    if "nc" not in _NC_CACHE:
        _NC_CACHE["nc"] = build_nc()
    return _NC_CACHE["nc"]


def _pack_smalls(inputs):
    f32 = lambda k: np.asarray(inputs[k], dtype=np.float32)
    pk3 = np.zeros((3, 258), dtype=np.float32)
    pk3[0:2, 0:256] = f32("W1").T
    pk3[2, 0:256] = f32("b1")
    pk3[0:2, 256] = f32("target")
    pk3[2, 256] = 1.0
    pk3[0:2, 257] = f32("bm")
    pk10 = np.zeros((10, 280), dtype=np.float32)
    pk10[:, 0:10] = f32("P").T
    pk10[:, 10:20] = f32("P")
    pk10[:, 20:22] = f32("Wm").T
    pk10[:, 22] = f32("D")
    pk10[:, 23] = f32("b2")
    pk10[:, 24:280] = f32("W2")
    return np.ascontiguousarray(pk3), np.ascontiguousarray(pk10)


def kernel(**inputs):
    nc = get_nc()
    eps = np.ascontiguousarray(
        np.asarray(inputs["eps"], dtype=np.float32).reshape(N_ARMS, F)
    )
    pk3, pk10 = _pack_smalls(inputs)
    in_maps = [
        {
            "pk3": pk3,
            "pk10": pk10,
            "eps": eps[i * ARMS_PER_CORE : (i + 1) * ARMS_PER_CORE],
        }
        for i in range(N_CORES)
    ]
    res = run_bass_kernel_spmd(nc, in_maps, core_ids=list(range(N_CORES)))
    out = np.concatenate([res.results[i]["out"] for i in range(N_CORES)], axis=0)
    return out.reshape(N_ARMS, 2, N_STEPS)


# revision 7
# speedup vs baseline: 1.3680x; 1.0490x over previous
"""Trainium2 Bass kernel for the arm-sampling rollout problem.

Math: the reference's 2048-step scan x <- x - (A@x)*dt with
A = P diag(exp(D)) P^-1 has the closed form
    hidden[k] = P diag(lam_i^k) P^-1 x0,   lam_i = 1 - dt*exp(D_i)
so actions^T[ch, k] = tanh(sum_i G[ch,i] * c_i * lam_i^k + bm[ch]) with
G = Wm @ P and c = P^-1 x0 (on-device Gauss-Jordan on [P^T | I]).
The output is the memory-bound broadcast
    out[arm, j] = 150*eps[arm, j] + 15000*act_flat[j]
over a [5000, 4096] array, 625 arms per core across 8 cores.

Key scheduling facts measured from NTFF profiles:
- A dma_start's descriptors (16KB per partition-row) are spread over
  the 16 DMA engines (26.8GB/s each) only while other transfers are
  co-resident in the queue's dispatch window; a transfer alone in the
  window crawls on ~1 engine. Each HWDGE queue also only admits ~5
  outstanding dma_starts (later triggers stall). So bulk I/O is split
  into ~1MB transfers across BOTH HWDGE queues (sync+scalar), with
  small trailing dummy transfers so no real transfer drains alone.
- PE fp32 matmul costs ~2.1us per 512 cols; bf16 ~0.55us. The action
  row is computed as ONE [8,512] fp32 matmul via the geometric-series
  split lam^(k+512j) = lam^(512j) * lam^k (lam^(512j) columns folded
  into 8 stacked lhsT columns), tanh'd to bf16, and broadcast 128-wide
  by selector-matrix bf16 matmuls (rhs = full [8,512] tile at base
  partition 0, so no cross-partition hop is needed).
- All small parameters, the identity/row-selector tables, and the
  step-index row are host-packed into three small partition-layout
  arrays (pk3, pk10a, pk10b), each loading as a handful of
  descriptors; gpsimd runs nothing but the tiny GJ column preps.
"""

import numpy as np

import concourse.bass as bass
import concourse.bacc as bacc
import concourse.mybir as mybir
import concourse.tile as tile
from concourse.bass_utils import run_bass_kernel_spmd

N_ARMS = 5000
N_STEPS = 2048
H = 10
F = 2 * N_STEPS  # 4096 flattened per-arm elements
N_CORES = 8
ARMS_PER_CORE = N_ARMS // N_CORES  # 625
FP = mybir.dt.float32
BF = mybir.dt.bfloat16

_NC_CACHE: dict = {}


def build_nc():
    AFT = mybir.ActivationFunctionType
    ALU = mybir.AluOpType

    nc = bacc.Bacc(
        "TRN2",
        target_bir_lowering=False,
        debug=False,
        enable_asserts=True,
        num_devices=N_CORES,
    )

    eps_d = nc.dram_tensor("eps", [ARMS_PER_CORE, F], FP, kind="ExternalInput")
    pk3_d = nc.dram_tensor("pk3", [3, 257], FP, kind="ExternalInput")
    pk10a_d = nc.dram_tensor("pk10a", [10, 136], FP, kind="ExternalInput")
    pk10b_d = nc.dram_tensor("pk10b", [10, 2304], FP, kind="ExternalInput")
    out_d = nc.dram_tensor("out", [ARMS_PER_CORE, F], FP, kind="ExternalOutput")
    dscr_d = nc.dram_tensor("dscr", [10, 280], FP, kind="Internal")

    TILE_ROWS = [(0, 128), (128, 256), (256, 384), (384, 512), (512, 625)]
    # 10 input row-groups of ~64 rows (~1MB each); first 5 on sync, rest scalar
    IN_GROUPS = [(r, min(r + 64, ARMS_PER_CORE)) for r in range(0, ARMS_PER_CORE, 64)]

    with tile.TileContext(nc) as tc:
        with (
            tc.tile_pool(name="sbc", bufs=1) as sbc,
            tc.tile_pool(name="sbgj", bufs=2) as sbgj,
            tc.tile_pool(name="sbeps", bufs=1) as sbeps,
            tc.tile_pool(name="psa", bufs=3, space=bass.MemorySpace.PSUM) as psa,
            tc.tile_pool(name="psbc", bufs=2, space=bass.MemorySpace.PSUM) as psbc,
            tc.tile_pool(name="psB", bufs=2, space=bass.MemorySpace.PSUM) as psB,
        ):
            # ---------- selector table built pre-input ------------------------
            # selm[:, 128r:128(r+1)] is the [8,128] lhsT that broadcasts row r
            seli = sbc.tile([8, 8 * 128], mybir.dt.int32, tag="seli")
            nc.gpsimd.iota(
                seli[:], pattern=[[-1, 8], [0, 128]], base=0, channel_multiplier=1
            )
            selm = sbc.tile([8, 8 * 128], BF, tag="selm")
            nc.vector.tensor_scalar(selm[:], seli[:], 0, None, ALU.is_equal)

            # ---------- sync HWDGE: pk3 + eps groups 0-4 + dummy --------------
            pk3 = sbc.tile([3, 257], FP, tag="pk3")
            nc.sync.dma_start(pk3[:], pk3_d.ap())
            eps_tiles = []
            for r0, r1 in TILE_ROWS:
                t = sbeps.tile([128, F], FP, tag="eps" + str(r0))
                eps_tiles.append((t, r0, r1 - r0))

            def load_group(eng, g0, g1):
                ti = g0 // 128
                t, r0, _ = eps_tiles[ti]
                eng.dma_start(t[g0 - r0 : g1 - r0, :], eps_d.ap()[g0:g1, :])

            for g0, g1 in IN_GROUPS[:5]:
                load_group(nc.sync, g0, g1)
            din1 = sbc.tile([4, F], FP, tag="din1")
            nc.sync.dma_start(din1[:], eps_d.ap()[0:4, :])

            # ---------- scalar HWDGE: pk10a/b + eps groups 5-9 + dummy --------
            pk10a = sbc.tile([10, 136], FP, tag="pk10a")
            nc.scalar.dma_start(pk10a[:], pk10a_d.ap())
            pk10b = sbc.tile([10, 2304], FP, tag="pk10b")
            nc.scalar.dma_start(pk10b[:], pk10b_d.ap())
            for g0, g1 in IN_GROUPS[5:]:
                load_group(nc.scalar, g0, g1)
            din2 = sbc.tile([4, F], FP, tag="din2")
            nc.scalar.dma_start(din2[:], eps_d.ap()[4:8, :])

            # pk10a: [:,0:10]=P^T [:,10:20]=P [:,20:22]=Wm^T [:,22]=D [:,23]=b2
            #        [:,24:34]=I10 [:,34:134]=oht [0:8,134]=bm8
            pT = pk10a[:, 0:10]
            p_nat = pk10a[:, 10:20]
            wmT = pk10a[:, 20:22]
            dcol = pk10a[:, 22:23]
            b2col = pk10a[:, 23:24]
            idm = pk10a[:, 24:34]
            oht = pk10a[:, 34:134]
            bm8 = pk10a[0:8, 134:135]
            # pk10b: [:,0:256]=W2, [:,256:2304]=kf (0..2047 each row)
            w2 = pk10b[:, 0:256]
            kf = pk10b[:, 256:2304]
            # pk3: rows0-1=W1^T, row2=b1; col256=[t0,t1,1]
            tgt1 = pk3[:, 256:257]

            # ---------- lam = 1 - 0.01*exp(D); vc = lam^k ---------------------
            es = sbc.tile([H, 1], FP, tag="es")
            nc.scalar.activation(es[:], dcol, AFT.Exp)
            lam = sbc.tile([H, 1], FP, tag="lam")
            nc.vector.tensor_scalar(lam[:], es[:], -0.01, 1.0, ALU.mult, ALU.add)
            lnl = sbc.tile([H, 1], FP, tag="lnl")
            nc.scalar.activation(lnl[:], lam[:], AFT.Ln)
            vc = sbc.tile([H, N_STEPS], FP, tag="vc")
            nc.scalar.activation(vc[:], kf, AFT.Exp, scale=lnl[:])

            # ---------- h = relu(W1 t + b1) via augmented-contraction mm ------
            hp0 = psa.tile([128, 1], FP, tag="mm")
            nc.tensor.matmul(hp0[:], pk3[:, 0:128], tgt1)
            hp1 = psa.tile([128, 1], FP, tag="mm")
            nc.tensor.matmul(hp1[:], pk3[:, 128:256], tgt1)
            h0 = sbc.tile([128, 1], FP, tag="h0")
            nc.scalar.activation(h0[:], hp0[:], AFT.Relu)
            h1 = sbc.tile([128, 1], FP, tag="h1")
            nc.scalar.activation(h1[:], hp1[:], AFT.Relu)

            # ---------- Gauss-Jordan on [P^T | I] -> Q = P^-T -----------------
            aug = sbgj.tile([H, 2 * H], FP, tag="aug")
            nc.vector.tensor_copy(aug[:, 0:H], pT)
            nc.vector.tensor_copy(aug[:, H : 2 * H], idm)
            for k in range(H):
                fn = sbgj.tile([H, 1], FP, tag="fn")
                nc.gpsimd.tensor_sub(fn[:], idm[:, k : k + 1], aug[:, k : k + 1])
                bc = psbc.tile([H, 2 * H], FP, tag="bc")
                nc.tensor.matmul(bc[:], oht[:, H * k : H * k + H], aug[:])
                piv = sbgj.tile([H, 1], FP, tag="piv")
                nc.vector.reciprocal(piv[:], bc[:, k : k + 1])
                S = sbgj.tile([H, 2 * H], FP, tag="S")
                nc.vector.tensor_scalar_mul(S[:], bc[:], piv[:])
                aug2 = sbgj.tile([H, 2 * H], FP, tag="aug")
                nc.vector.scalar_tensor_tensor(
                    aug2[:], S[:], fn[:], aug[:], ALU.mult, ALU.add
                )
                aug = aug2

            # ---------- W2^T, x0, c, G^T*c ------------------------------------
            w2tp0 = psa.tile([128, H], FP, tag="mm")
            nc.tensor.matmul(w2tp0[:], w2[:, 0:128], idm, is_transpose=True)
            w2tp1 = psa.tile([128, H], FP, tag="mm")
            nc.tensor.matmul(w2tp1[:], w2[:, 128:256], idm, is_transpose=True)
            w2t0 = sbc.tile([128, H], FP, tag="w2t0")
            nc.vector.tensor_copy(w2t0[:], w2tp0[:])
            w2t1 = sbc.tile([128, H], FP, tag="w2t1")
            nc.vector.tensor_copy(w2t1[:], w2tp1[:])
            x0p = psa.tile([H, 1], FP, tag="mm")
            nc.tensor.matmul(x0p[:], w2t0[:], h0[:], start=True, stop=False)
            nc.tensor.matmul(x0p[:], w2t1[:], h1[:], start=False, stop=True)
            x0s = sbc.tile([H, 1], FP, tag="x0s")
            nc.scalar.activation(x0s[:], x0p[:], AFT.Identity, bias=b2col, scale=1.0)
            gtp = psa.tile([H, 2], FP, tag="mm")
            nc.tensor.matmul(gtp[:], p_nat, wmT)
            cp = psa.tile([H, 1], FP, tag="mm")
            nc.tensor.matmul(cp[:], aug[:, H : 2 * H], x0s[:])
            gts = sbc.tile([H, 2], FP, tag="gts")
            nc.vector.tensor_scalar_mul(gts[:], gtp[:], cp[:, 0:1])

            # ---------- actions: ONE [8,512] mm via lam^(512j) folding --------
            # gstack[:, 2j+ch] = gts[:, ch] * lam^(512j); vc cols 0,512,1024,1536
            gstack = sbc.tile([H, 8], FP, tag="gstack")
            for j in range(4):
                nc.vector.tensor_scalar_mul(
                    gstack[:, 2 * j : 2 * j + 2], gts[:], vc[:, 512 * j : 512 * j + 1]
                )
            pre8 = psa.tile([8, 512], FP, tag="mm")
            nc.tensor.matmul(pre8[:], gstack[:], vc[:, 0:512])
            ats8 = sbc.tile([8, 512], BF, tag="ats8")
            nc.scalar.activation(ats8[:], pre8[:], AFT.Tanh, bias=bm8, scale=1.0)

            # ---------- B[p, 2t+ch] = 15000*row_(2j+ch)(ats8) broadcast -------
            Bsb = sbc.tile([128, F], FP, tag="B")
            B3 = Bsb[:].rearrange("p (t m) -> p t m", m=2)
            for r in range(8):
                j, ch = r // 2, r % 2
                bp = psB.tile([128, 512], FP, tag="B")
                nc.tensor.matmul(bp[:], selm[:, 128 * r : 128 * (r + 1)], ats8[:])
                nc.scalar.activation(
                    B3[:, 512 * j : 512 * (j + 1), ch : ch + 1],
                    bp[:, :, None],
                    AFT.Copy,
                    scale=15000.0,
                )

            # ---------- main: out = 150*eps + B; h0s->sync, h1s->scalar -------
            HW = F // 2
            for hh in range(2):
                c0, c1 = hh * HW, (hh + 1) * HW
                eng = nc.sync if hh == 0 else nc.scalar
                for t, r0, pt in eps_tiles:
                    nc.vector.scalar_tensor_tensor(
                        t[0:pt, c0:c1],
                        t[0:pt, c0:c1],
                        150.0,
                        Bsb[0:pt, c0:c1],
                        ALU.mult,
                        ALU.add,
                    )
                    eng.dma_start(out_d.ap()[r0 : r0 + pt, c0:c1], t[0:pt, c0:c1])
                # dummy companion so the final window entry is never alone
                eng.dma_start(
                    dscr_d.ap()[4 * hh : 4 * hh + 4, 0:136], pk10a[4:8, :]
                )

    nc.compile()
    return nc


def get_nc():
    if "nc" not in _NC_CACHE:
        _NC_CACHE["nc"] = build_nc()
    return _NC_CACHE["nc"]


def _pack_smalls(inputs):
    f32 = lambda k: np.asarray(inputs[k], dtype=np.float32)
    pk3 = np.zeros((3, 257), dtype=np.float32)
    pk3[0:2, 0:256] = f32("W1").T
    pk3[2, 0:256] = f32("b1")
    pk3[0:2, 256] = f32("target")
    pk3[2, 256] = 1.0
    pk10a = np.zeros((10, 136), dtype=np.float32)
    pk10a[:, 0:10] = f32("P").T
    pk10a[:, 10:20] = f32("P")
    pk10a[:, 20:22] = f32("Wm").T
    pk10a[:, 22] = f32("D")
    pk10a[:, 23] = f32("b2")
    pk10a[:, 24:34] = np.eye(10, dtype=np.float32)
    pk10a[:, 34:134] = np.repeat(np.eye(10, dtype=np.float32), 10, axis=1)
    pk10a[0:8, 134] = np.tile(f32("bm"), 4)
    pk10b = np.zeros((10, 2304), dtype=np.float32)
    pk10b[:, 0:256] = f32("W2")
    pk10b[:, 256:2304] = np.arange(N_STEPS, dtype=np.float32)[None, :]
    return (
        np.ascontiguousarray(pk3),
        np.ascontiguousarray(pk10a),
        np.ascontiguousarray(pk10b),
    )


def kernel(**inputs):
    nc = get_nc()
    eps = np.ascontiguousarray(
        np.asarray(inputs["eps"], dtype=np.float32).reshape(N_ARMS, F)
    )
    pk3, pk10a, pk10b = _pack_smalls(inputs)
    in_maps = [
        {
            "pk3": pk3,
            "pk10a": pk10a,
            "pk10b": pk10b,
            "eps": eps[i * ARMS_PER_CORE : (i + 1) * ARMS_PER_CORE],
        }
        for i in range(N_CORES)
    ]
    res = run_bass_kernel_spmd(nc, in_maps, core_ids=list(range(N_CORES)))
    out = np.concatenate([res.results[i]["out"] for i in range(N_CORES)], axis=0)
    return out.reshape(N_ARMS, 2, N_STEPS)


# revision 10
# speedup vs baseline: 1.3930x; 1.0183x over previous
"""Trainium2 Bass kernel for the arm-sampling rollout problem.

Math: the reference's 2048-step scan x <- x - (A@x)*dt with
A = P diag(exp(D)) P^-1 has the closed form
    hidden[k] = P diag(lam_i^k) P^-1 x0,   lam_i = 1 - dt*exp(D_i)
so actions^T[ch, k] = tanh(sum_i G[ch,i] * c_i * lam_i^k + bm[ch]) with
G = Wm @ P and c = P^-1 x0 (on-device Gauss-Jordan on [P^T | I]).
The output is the memory-bound broadcast
    out[arm, j] = 150*eps[arm, j] + 15000*act_flat[j]
over a [5000, 4096] array, 625 arms per core across 8 cores.

Key scheduling facts measured from NTFF profiles:
- A dma_start's descriptors (16KB per partition-row) are spread over
  the 16 DMA engines (26.8GB/s each) only while other transfers are
  co-resident in the queue's dispatch window; a transfer alone in the
  window crawls on ~1 engine. Each HWDGE queue also only admits ~5
  outstanding dma_starts (later triggers stall). So bulk I/O is split
  into ~1MB transfers across BOTH HWDGE queues (sync+scalar), with
  small trailing dummy transfers so no real transfer drains alone.
- PE fp32 matmul costs ~2.1us per 512 cols; bf16 ~0.55us. The action
  row is computed as ONE [8,512] fp32 matmul via the geometric-series
  split lam^(k+512j) = lam^(512j) * lam^k (lam^(512j) columns folded
  into 8 stacked lhsT columns), tanh'd to bf16, and broadcast 128-wide
  by selector-matrix bf16 matmuls (rhs = full [8,512] tile at base
  partition 0, so no cross-partition hop is needed).
- All small parameters, the identity/row-selector tables, and the
  step-index row are host-packed into three small partition-layout
  arrays (pk3, pk10a, pk10b), each loading as a handful of
  descriptors; gpsimd runs nothing but the tiny GJ column preps.
"""

import numpy as np

import concourse.bass as bass
import concourse.bacc as bacc
import concourse.mybir as mybir
import concourse.tile as tile
from concourse.bass_utils import run_bass_kernel_spmd

N_ARMS = 5000
N_STEPS = 2048
H = 10
F = 2 * N_STEPS  # 4096 flattened per-arm elements
N_CORES = 8
ARMS_PER_CORE = N_ARMS // N_CORES  # 625
FP = mybir.dt.float32
BF = mybir.dt.bfloat16

_NC_CACHE: dict = {}


def build_nc():
    AFT = mybir.ActivationFunctionType
    ALU = mybir.AluOpType

    nc = bacc.Bacc(
        "TRN2",
        target_bir_lowering=False,
        debug=False,
        enable_asserts=True,
        num_devices=N_CORES,
    )

    eps_d = nc.dram_tensor("eps", [ARMS_PER_CORE, F], FP, kind="ExternalInput")
    pk3_d = nc.dram_tensor("pk3", [3, 257], FP, kind="ExternalInput")
    pk10a_d = nc.dram_tensor("pk10a", [10, 136], FP, kind="ExternalInput")
    pk10b_d = nc.dram_tensor("pk10b", [10, 2304], FP, kind="ExternalInput")
    out_d = nc.dram_tensor("out", [ARMS_PER_CORE, F], FP, kind="ExternalOutput")
    dscr_d = nc.dram_tensor("dscr", [32, F], FP, kind="Internal")

    TILE_ROWS = [(0, 128), (128, 256), (256, 384), (384, 512), (512, 625)]
    # big input transfers: 4 full tiles, last tile split + tapered dummy tail
    IN_GROUPS = [(0, 128), (128, 256), (256, 384), (384, 512), (512, 569), (569, 625)]

    with tile.TileContext(nc) as tc:
        with (
            tc.tile_pool(name="sbc", bufs=1) as sbc,
            tc.tile_pool(name="sbgj", bufs=2) as sbgj,
            tc.tile_pool(name="sbeps", bufs=1) as sbeps,
            tc.tile_pool(name="psa", bufs=3, space=bass.MemorySpace.PSUM) as psa,
            tc.tile_pool(name="psbc", bufs=2, space=bass.MemorySpace.PSUM) as psbc,
            tc.tile_pool(name="psB", bufs=2, space=bass.MemorySpace.PSUM) as psB,
        ):
            # ---------- selector table built pre-input ------------------------
            # selm[:, 128r:128(r+1)] is the [8,128] lhsT that broadcasts row r
            seli = sbc.tile([8, 8 * 128], mybir.dt.int32, tag="seli")
            nc.gpsimd.iota(
                seli[:], pattern=[[-1, 8], [0, 128]], base=0, channel_multiplier=1
            )
            selm = sbc.tile([8, 8 * 128], BF, tag="selm")
            nc.vector.tensor_scalar(selm[:], seli[:], 0, None, ALU.is_equal)

            # ---------- sync HWDGE: pk3 + eps groups 0-4 + dummy --------------
            pk3 = sbc.tile([3, 257], FP, tag="pk3")
            nc.sync.dma_start(pk3[:], pk3_d.ap())
            eps_tiles = []
            for r0, r1 in TILE_ROWS:
                t = sbeps.tile([128, F], FP, tag="eps" + str(r0))
                eps_tiles.append((t, r0, r1 - r0))

            def load_group(eng, g0, g1):
                ti = g0 // 128
                t, r0, _ = eps_tiles[ti]
                eng.dma_start(t[g0 - r0 : g1 - r0, :], eps_d.ap()[g0:g1, :])

            for g0, g1 in IN_GROUPS:
                load_group(nc.sync, g0, g1)
            # ~0.5MB dummy rides with the input tail so it drains in spread mode
            din1 = sbc.tile([32, F], FP, tag="din1")
            nc.sync.dma_start(din1[:], eps_d.ap()[0:32, :])

            # ---------- scalar HWDGE: pk10a/b (outputs come later) ------------
            pk10a = sbc.tile([10, 136], FP, tag="pk10a")
            nc.scalar.dma_start(pk10a[:], pk10a_d.ap())
            pk10b = sbc.tile([10, 2304], FP, tag="pk10b")
            nc.scalar.dma_start(pk10b[:], pk10b_d.ap())

            # pk10a: [:,0:10]=P^T [:,10:20]=P [:,20:22]=Wm^T [:,22]=D [:,23]=b2
            #        [:,24:34]=I10 [:,34:134]=oht [0:8,134]=bm8
            pT = pk10a[:, 0:10]
            p_nat = pk10a[:, 10:20]
            wmT = pk10a[:, 20:22]
            dcol = pk10a[:, 22:23]
            b2col = pk10a[:, 23:24]
            idm = pk10a[:, 24:34]
            oht = pk10a[:, 34:134]
            bm8 = pk10a[0:8, 134:135]
            # pk10b: [:,0:256]=W2, [:,256:2304]=kf (0..2047 each row)
            w2 = pk10b[:, 0:256]
            kf = pk10b[:, 256:2304]
            # pk3: rows0-1=W1^T, row2=b1; col256=[t0,t1,1]
            tgt1 = pk3[:, 256:257]

            # ---------- lam = 1 - 0.01*exp(D); vc = lam^k ---------------------
            es = sbc.tile([H, 1], FP, tag="es")
            nc.scalar.activation(es[:], dcol, AFT.Exp)
            lam = sbc.tile([H, 1], FP, tag="lam")
            nc.vector.tensor_scalar(lam[:], es[:], -0.01, 1.0, ALU.mult, ALU.add)
            lnl = sbc.tile([H, 1], FP, tag="lnl")
            nc.scalar.activation(lnl[:], lam[:], AFT.Ln)
            vc = sbc.tile([H, N_STEPS], FP, tag="vc")
            nc.scalar.activation(vc[:], kf, AFT.Exp, scale=lnl[:])

            # ---------- h = relu(W1 t + b1) via augmented-contraction mm ------
            hp0 = psa.tile([128, 1], FP, tag="mm")
            nc.tensor.matmul(hp0[:], pk3[:, 0:128], tgt1)
            hp1 = psa.tile([128, 1], FP, tag="mm")
            nc.tensor.matmul(hp1[:], pk3[:, 128:256], tgt1)
            h0 = sbc.tile([128, 1], FP, tag="h0")
            nc.scalar.activation(h0[:], hp0[:], AFT.Relu)
            h1 = sbc.tile([128, 1], FP, tag="h1")
            nc.scalar.activation(h1[:], hp1[:], AFT.Relu)

            # ---------- Gauss-Jordan on [P^T | I] -> Q = P^-T -----------------
            aug = sbgj.tile([H, 2 * H], FP, tag="aug")
            nc.vector.tensor_copy(aug[:, 0:H], pT)
            nc.vector.tensor_copy(aug[:, H : 2 * H], idm)
            for k in range(H):
                fn = sbgj.tile([H, 1], FP, tag="fn")
                nc.gpsimd.tensor_sub(fn[:], idm[:, k : k + 1], aug[:, k : k + 1])
                bc = psbc.tile([H, 2 * H], FP, tag="bc")
                nc.tensor.matmul(bc[:], oht[:, H * k : H * k + H], aug[:])
                piv = sbgj.tile([H, 1], FP, tag="piv")
                nc.vector.reciprocal(piv[:], bc[:, k : k + 1])
                S = sbgj.tile([H, 2 * H], FP, tag="S")
                nc.vector.tensor_scalar_mul(S[:], bc[:], piv[:])
                aug2 = sbgj.tile([H, 2 * H], FP, tag="aug")
                nc.vector.scalar_tensor_tensor(
                    aug2[:], S[:], fn[:], aug[:], ALU.mult, ALU.add
                )
                aug = aug2

            # ---------- W2^T, x0, c, G^T*c ------------------------------------
            w2tp0 = psa.tile([128, H], FP, tag="mm")
            nc.tensor.matmul(w2tp0[:], w2[:, 0:128], idm, is_transpose=True)
            w2tp1 = psa.tile([128, H], FP, tag="mm")
            nc.tensor.matmul(w2tp1[:], w2[:, 128:256], idm, is_transpose=True)
            w2t0 = sbc.tile([128, H], FP, tag="w2t0")
            nc.vector.tensor_copy(w2t0[:], w2tp0[:])
            w2t1 = sbc.tile([128, H], FP, tag="w2t1")
            nc.vector.tensor_copy(w2t1[:], w2tp1[:])
            x0p = psa.tile([H, 1], FP, tag="mm")
            nc.tensor.matmul(x0p[:], w2t0[:], h0[:], start=True, stop=False)
            nc.tensor.matmul(x0p[:], w2t1[:], h1[:], start=False, stop=True)
            x0s = sbc.tile([H, 1], FP, tag="x0s")
            nc.scalar.activation(x0s[:], x0p[:], AFT.Identity, bias=b2col, scale=1.0)
            gtp = psa.tile([H, 2], FP, tag="mm")
            nc.tensor.matmul(gtp[:], p_nat, wmT)
            cp = psa.tile([H, 1], FP, tag="mm")
            nc.tensor.matmul(cp[:], aug[:, H : 2 * H], x0s[:])
            gts = sbc.tile([H, 2], FP, tag="gts")
            nc.vector.tensor_scalar_mul(gts[:], gtp[:], cp[:, 0:1])

            # ---------- actions: ONE [8,512] mm via lam^(512j) folding --------
            # gstack[:, 2j+ch] = gts[:, ch] * lam^(512j); vc cols 0,512,1024,1536
            gstack = sbc.tile([H, 8], FP, tag="gstack")
            for j in range(4):
                nc.vector.tensor_scalar_mul(
                    gstack[:, 2 * j : 2 * j + 2], gts[:], vc[:, 512 * j : 512 * j + 1]
                )
            pre8 = psa.tile([8, 512], FP, tag="mm")
            nc.tensor.matmul(pre8[:], gstack[:], vc[:, 0:512])
            ats8 = sbc.tile([8, 512], BF, tag="ats8")
            nc.scalar.activation(ats8[:], pre8[:], AFT.Tanh, bias=bm8, scale=1.0)

            # ---------- B[p, 2t+ch] = 15000*row_(2j+ch)(ats8) broadcast -------
            Bsb = sbc.tile([128, F], FP, tag="B")
            B3 = Bsb[:].rearrange("p (t m) -> p t m", m=2)
            for r in range(8):
                j, ch = r // 2, r % 2
                bp = psB.tile([128, 512], FP, tag="B")
                nc.tensor.matmul(bp[:], selm[:, 128 * r : 128 * (r + 1)], ats8[:])
                nc.scalar.activation(
                    B3[:, 512 * j : 512 * (j + 1), ch : ch + 1],
                    bp[:, :, None],
                    AFT.Copy,
                    scale=15000.0,
                )

            # ---------- main: out = 150*eps + B; full-tile outputs on scalar --
            for t, r0, pt in eps_tiles:
                nc.vector.scalar_tensor_tensor(
                    t[0:pt, :], t[0:pt, :], 150.0, Bsb[0:pt, :], ALU.mult, ALU.add
                )
                nc.scalar.dma_start(out_d.ap()[r0 : r0 + pt, :], t[0:pt, :])
            # ~0.5MB dummy rides with the output tail (spread-mode companion)
            nc.scalar.dma_start(dscr_d.ap(), Bsb[0:32, :])

    nc.compile()
    return nc


def get_nc():
    if "nc" not in _NC_CACHE:
        _NC_CACHE["nc"] = build_nc()
    return _NC_CACHE["nc"]


def _pack_smalls(inputs):
    f32 = lambda k: np.asarray(inputs[k], dtype=np.float32)
    pk3 = np.zeros((3, 257), dtype=np.float32)
    pk3[0:2, 0:256] = f32("W1").T
    pk3[2, 0:256] = f32("b1")
    pk3[0:2, 256] = f32("target")
    pk3[2, 256] = 1.0
    pk10a = np.zeros((10, 136), dtype=np.float32)
    pk10a[:, 0:10] = f32("P").T
    pk10a[:, 10:20] = f32("P")
    pk10a[:, 20:22] = f32("Wm").T
    pk10a[:, 22] = f32("D")
    pk10a[:, 23] = f32("b2")
    pk10a[:, 24:34] = np.eye(10, dtype=np.float32)
    pk10a[:, 34:134] = np.repeat(np.eye(10, dtype=np.float32), 10, axis=1)
    pk10a[0:8, 134] = np.tile(f32("bm"), 4)
    pk10b = np.zeros((10, 2304), dtype=np.float32)
    pk10b[:, 0:256] = f32("W2")
    pk10b[:, 256:2304] = np.arange(N_STEPS, dtype=np.float32)[None, :]
    return (
        np.ascontiguousarray(pk3),
        np.ascontiguousarray(pk10a),
        np.ascontiguousarray(pk10b),
    )


def kernel(**inputs):
    nc = get_nc()
    eps = np.ascontiguousarray(
        np.asarray(inputs["eps"], dtype=np.float32).reshape(N_ARMS, F)
    )
    pk3, pk10a, pk10b = _pack_smalls(inputs)
    in_maps = [
        {
            "pk3": pk3,
            "pk10a": pk10a,
            "pk10b": pk10b,
            "eps": eps[i * ARMS_PER_CORE : (i + 1) * ARMS_PER_CORE],
        }
        for i in range(N_CORES)
    ]
    res = run_bass_kernel_spmd(nc, in_maps, core_ids=list(range(N_CORES)))
    out = np.concatenate([res.results[i]["out"] for i in range(N_CORES)], axis=0)
    return out.reshape(N_ARMS, 2, N_STEPS)


# revision 12
# speedup vs baseline: 1.8323x; 1.3154x over previous
"""Trainium2 Bass kernel for the arm-sampling rollout problem.

Math: the reference's 2048-step scan x <- x - (A@x)*dt with
A = P diag(exp(D)) P^-1 has the closed form
    hidden[k] = P diag(lam_i^k) P^-1 x0,   lam_i = 1 - dt*exp(D_i)
so actions^T[ch, k] = tanh(sum_i G[ch,i] * c_i * lam_i^k + bm[ch]) with
G = Wm @ P and c = P^-1 x0 (on-device Gauss-Jordan on [P^T | I]).
The output is the memory-bound broadcast
    out[arm, j] = 150*eps[arm, j] + 15000*act_flat[j]
over a [5000, 4096] array, 625 arms per core across 8 cores.

Key scheduling facts measured from NTFF profiles:
- A dma_start's descriptors (16KB per partition-row) are spread over
  the 16 DMA engines (26.8GB/s each) only while other transfers are
  co-resident in the queue's dispatch window; a transfer alone in the
  window crawls on ~1 engine. Each HWDGE queue also only admits ~5
  outstanding dma_starts (later triggers stall). So bulk I/O is split
  into ~1MB transfers across BOTH HWDGE queues (sync+scalar), with
  small trailing dummy transfers so no real transfer drains alone.
- PE fp32 matmul costs ~2.1us per 512 cols; bf16 ~0.55us. The action
  row is computed as ONE [8,512] fp32 matmul via the geometric-series
  split lam^(k+512j) = lam^(512j) * lam^k (lam^(512j) columns folded
  into 8 stacked lhsT columns), tanh'd to bf16, and broadcast 128-wide
  by selector-matrix bf16 matmuls (rhs = full [8,512] tile at base
  partition 0, so no cross-partition hop is needed).
- All small parameters, the identity/row-selector tables, and the
  step-index row are host-packed into three small partition-layout
  arrays (pk3, pk10a, pk10b), each loading as a handful of
  descriptors; gpsimd runs nothing but the tiny GJ column preps.
"""

import numpy as np

import concourse.bass as bass
import concourse.bacc as bacc
import concourse.mybir as mybir
import concourse.tile as tile
from concourse.bass_utils import run_bass_kernel_spmd

N_ARMS = 5000
N_STEPS = 2048
H = 10
F = 2 * N_STEPS  # 4096 flattened per-arm elements
N_CORES = 8
ARMS_PER_CORE = N_ARMS // N_CORES  # 625
FP = mybir.dt.float32
BF = mybir.dt.bfloat16

_NC_CACHE: dict = {}


def build_nc():
    AFT = mybir.ActivationFunctionType
    ALU = mybir.AluOpType

    nc = bacc.Bacc(
        "TRN2",
        target_bir_lowering=False,
        debug=False,
        enable_asserts=True,
        num_devices=N_CORES,
    )

    eps_d = nc.dram_tensor("eps", [ARMS_PER_CORE, F], FP, kind="ExternalInput")
    pk3_d = nc.dram_tensor("pk3", [3, 257], FP, kind="ExternalInput")
    pk10a_d = nc.dram_tensor("pk10a", [10, 136], FP, kind="ExternalInput")
    pk10b_d = nc.dram_tensor("pk10b", [10, 2304], FP, kind="ExternalInput")
    out_d = nc.dram_tensor("out", [ARMS_PER_CORE, F], FP, kind="ExternalOutput")
    dscr_d = nc.dram_tensor("dscr", [32, F], FP, kind="Internal")

    TILE_ROWS = [(0, 128), (128, 256), (256, 384), (384, 512), (512, 625)]
    # big input transfers up front; the tail is a tight burst of equal small
    # pieces (+ dummy) so the final dispatch window spreads and co-finishes
    IN_GROUPS = [
        (0, 128), (128, 256), (256, 384), (384, 512),
        (512, 550), (550, 588), (588, 625),
    ]

    with tile.TileContext(nc) as tc:
        with (
            tc.tile_pool(name="sbc", bufs=1) as sbc,
            tc.tile_pool(name="sbgj", bufs=2) as sbgj,
            tc.tile_pool(name="sbeps", bufs=1) as sbeps,
            tc.tile_pool(name="psa", bufs=3, space=bass.MemorySpace.PSUM) as psa,
            tc.tile_pool(name="psbc", bufs=2, space=bass.MemorySpace.PSUM) as psbc,
            tc.tile_pool(name="psB", bufs=2, space=bass.MemorySpace.PSUM) as psB,
        ):
            # ---------- selector table built pre-input ------------------------
            # selm[:, 128r:128(r+1)] is the [8,128] lhsT that broadcasts row r
            seli = sbc.tile([8, 8 * 128], mybir.dt.int32, tag="seli")
            nc.gpsimd.iota(
                seli[:], pattern=[[-1, 8], [0, 128]], base=0, channel_multiplier=1
            )
            selm = sbc.tile([8, 8 * 128], BF, tag="selm")
            nc.vector.tensor_scalar(selm[:], seli[:], 0, None, ALU.is_equal)

            # ---------- sync HWDGE: pk3 + eps groups 0-4 + dummy --------------
            pk3 = sbc.tile([3, 257], FP, tag="pk3")
            nc.sync.dma_start(pk3[:], pk3_d.ap())
            eps_tiles = []
            for r0, r1 in TILE_ROWS:
                t = sbeps.tile([128, F], FP, tag="eps" + str(r0))
                eps_tiles.append((t, r0, r1 - r0))

            def load_group(eng, g0, g1):
                ti = g0 // 128
                t, r0, _ = eps_tiles[ti]
                eng.dma_start(t[g0 - r0 : g1 - r0, :], eps_d.ap()[g0:g1, :])

            for g0, g1 in IN_GROUPS:
                load_group(nc.sync, g0, g1)
            # ~0.5MB dummy rides with the input tail so it drains in spread mode
            din1 = sbc.tile([32, F], FP, tag="din1")
            nc.sync.dma_start(din1[:], eps_d.ap()[0:32, :])

            # ---------- scalar HWDGE: pk10a/b (outputs come later) ------------
            pk10a = sbc.tile([10, 136], FP, tag="pk10a")
            nc.scalar.dma_start(pk10a[:], pk10a_d.ap())
            pk10b = sbc.tile([10, 2304], FP, tag="pk10b")
            nc.scalar.dma_start(pk10b[:], pk10b_d.ap())

            # pk10a: [:,0:10]=P^T [:,10:20]=P [:,20:22]=Wm^T [:,22]=D [:,23]=b2
            #        [:,24:34]=I10 [:,34:134]=oht [0:8,134]=bm8
            pT = pk10a[:, 0:10]
            p_nat = pk10a[:, 10:20]
            wmT = pk10a[:, 20:22]
            dcol = pk10a[:, 22:23]
            b2col = pk10a[:, 23:24]
            idm = pk10a[:, 24:34]
            oht = pk10a[:, 34:134]
            bm8 = pk10a[0:8, 134:135]
            # pk10b: [:,0:256]=W2, [:,256:2304]=kf (0..2047 each row)
            w2 = pk10b[:, 0:256]
            kf = pk10b[:, 256:2304]
            # pk3: rows0-1=W1^T, row2=b1; col256=[t0,t1,1]
            tgt1 = pk3[:, 256:257]

            # ---------- lam = 1 - 0.01*exp(D); vc = lam^k ---------------------
            es = sbc.tile([H, 1], FP, tag="es")
            nc.scalar.activation(es[:], dcol, AFT.Exp)
            lam = sbc.tile([H, 1], FP, tag="lam")
            nc.vector.tensor_scalar(lam[:], es[:], -0.01, 1.0, ALU.mult, ALU.add)
            lnl = sbc.tile([H, 1], FP, tag="lnl")
            nc.scalar.activation(lnl[:], lam[:], AFT.Ln)
            vc = sbc.tile([H, N_STEPS], FP, tag="vc")
            nc.scalar.activation(vc[:], kf, AFT.Exp, scale=lnl[:])

            # ---------- h = relu(W1 t + b1) via augmented-contraction mm ------
            hp0 = psa.tile([128, 1], FP, tag="mm")
            nc.tensor.matmul(hp0[:], pk3[:, 0:128], tgt1)
            hp1 = psa.tile([128, 1], FP, tag="mm")
            nc.tensor.matmul(hp1[:], pk3[:, 128:256], tgt1)
            h0 = sbc.tile([128, 1], FP, tag="h0")
            nc.scalar.activation(h0[:], hp0[:], AFT.Relu)
            h1 = sbc.tile([128, 1], FP, tag="h1")
            nc.scalar.activation(h1[:], hp1[:], AFT.Relu)

            # ---------- Gauss-Jordan on [P^T | I] -> Q = P^-T -----------------
            aug = sbgj.tile([H, 2 * H], FP, tag="aug")
            nc.vector.tensor_copy(aug[:, 0:H], pT)
            nc.vector.tensor_copy(aug[:, H : 2 * H], idm)
            for k in range(H):
                fn = sbgj.tile([H, 1], FP, tag="fn")
                nc.gpsimd.tensor_sub(fn[:], idm[:, k : k + 1], aug[:, k : k + 1])
                bc = psbc.tile([H, 2 * H], FP, tag="bc")
                nc.tensor.matmul(bc[:], oht[:, H * k : H * k + H], aug[:])
                piv = sbgj.tile([H, 1], FP, tag="piv")
                nc.vector.reciprocal(piv[:], bc[:, k : k + 1])
                S = sbgj.tile([H, 2 * H], FP, tag="S")
                nc.vector.tensor_scalar_mul(S[:], bc[:], piv[:])
                aug2 = sbgj.tile([H, 2 * H], FP, tag="aug")
                nc.vector.scalar_tensor_tensor(
                    aug2[:], S[:], fn[:], aug[:], ALU.mult, ALU.add
                )
                aug = aug2

            # ---------- W2^T, x0, c, G^T*c ------------------------------------
            w2tp0 = psa.tile([128, H], FP, tag="mm")
            nc.tensor.matmul(w2tp0[:], w2[:, 0:128], idm, is_transpose=True)
            w2tp1 = psa.tile([128, H], FP, tag="mm")
            nc.tensor.matmul(w2tp1[:], w2[:, 128:256], idm, is_transpose=True)
            w2t0 = sbc.tile([128, H], FP, tag="w2t0")
            nc.vector.tensor_copy(w2t0[:], w2tp0[:])
            w2t1 = sbc.tile([128, H], FP, tag="w2t1")
            nc.vector.tensor_copy(w2t1[:], w2tp1[:])
            x0p = psa.tile([H, 1], FP, tag="mm")
            nc.tensor.matmul(x0p[:], w2t0[:], h0[:], start=True, stop=False)
            nc.tensor.matmul(x0p[:], w2t1[:], h1[:], start=False, stop=True)
            x0s = sbc.tile([H, 1], FP, tag="x0s")
            nc.scalar.activation(x0s[:], x0p[:], AFT.Identity, bias=b2col, scale=1.0)
            gtp = psa.tile([H, 2], FP, tag="mm")
            nc.tensor.matmul(gtp[:], p_nat, wmT)
            cp = psa.tile([H, 1], FP, tag="mm")
            nc.tensor.matmul(cp[:], aug[:, H : 2 * H], x0s[:])
            gts = sbc.tile([H, 2], FP, tag="gts")
            nc.vector.tensor_scalar_mul(gts[:], gtp[:], cp[:, 0:1])

            # ---------- actions: ONE [8,512] mm via lam^(512j) folding --------
            # gstack[:, 2j+ch] = gts[:, ch] * lam^(512j); vc cols 0,512,1024,1536
            gstack = sbc.tile([H, 8], FP, tag="gstack")
            for j in range(4):
                nc.vector.tensor_scalar_mul(
                    gstack[:, 2 * j : 2 * j + 2], gts[:], vc[:, 512 * j : 512 * j + 1]
                )
            pre8 = psa.tile([8, 512], FP, tag="mm")
            nc.tensor.matmul(pre8[:], gstack[:], vc[:, 0:512])
            ats8 = sbc.tile([8, 512], BF, tag="ats8")
            nc.scalar.activation(ats8[:], pre8[:], AFT.Tanh, bias=bm8, scale=1.0)

            # ---------- B[p, 2t+ch] = 15000*row_(2j+ch)(ats8) broadcast -------
            # copies split scalar (ch0) / vector (ch1) so B's halves complete
            # as the bcast matmuls stream out of the PE
            Bsb = sbc.tile([128, F], FP, tag="B")
            B3 = Bsb[:].rearrange("p (t m) -> p t m", m=2)
            for r in range(8):
                j, ch = r // 2, r % 2
                bp = psB.tile([128, 512], FP, tag="B")
                nc.tensor.matmul(bp[:], selm[:, 128 * r : 128 * (r + 1)], ats8[:])
                dst = B3[:, 512 * j : 512 * (j + 1), ch : ch + 1]
                if ch == 0:
                    nc.scalar.activation(dst, bp[:, :, None], AFT.Copy, scale=15000.0)
                else:
                    nc.vector.tensor_scalar(
                        dst, bp[:, :, None], 15000.0, None, ALU.mult
                    )

            # ---------- main: out = 150*eps + B (half-col STTs so work can ---
            # start on B's first half); one big output DMA per tile, and the
            # last tile's output goes out as a tight burst of 2 pieces + dummy
            HW = F // 2
            for t, r0, pt in eps_tiles:
                for hh in range(2):
                    c0, c1 = hh * HW, (hh + 1) * HW
                    nc.vector.scalar_tensor_tensor(
                        t[0:pt, c0:c1],
                        t[0:pt, c0:c1],
                        150.0,
                        Bsb[0:pt, c0:c1],
                        ALU.mult,
                        ALU.add,
                    )
                if pt == 128:
                    nc.scalar.dma_start(out_d.ap()[r0 : r0 + pt, :], t[0:pt, :])
                else:
                    nc.scalar.dma_start(out_d.ap()[r0 : r0 + 57, :], t[0:57, :])
                    nc.scalar.dma_start(out_d.ap()[r0 + 57 : r0 + pt, :], t[57:pt, :])
                    nc.scalar.dma_start(dscr_d.ap(), Bsb[0:32, :])

    nc.compile()
    return nc


def get_nc():
    if "nc" not in _NC_CACHE:
        _NC_CACHE["nc"] = build_nc()
    return _NC_CACHE["nc"]


def _pack_smalls(inputs):
    f32 = lambda k: np.asarray(inputs[k], dtype=np.float32)
    pk3 = np.zeros((3, 257), dtype=np.float32)
    pk3[0:2, 0:256] = f32("W1").T
    pk3[2, 0:256] = f32("b1")
    pk3[0:2, 256] = f32("target")
    pk3[2, 256] = 1.0
    pk10a = np.zeros((10, 136), dtype=np.float32)
    pk10a[:, 0:10] = f32("P").T
    pk10a[:, 10:20] = f32("P")
    pk10a[:, 20:22] = f32("Wm").T
    pk10a[:, 22] = f32("D")
    pk10a[:, 23] = f32("b2")
    pk10a[:, 24:34] = np.eye(10, dtype=np.float32)
    pk10a[:, 34:134] = np.repeat(np.eye(10, dtype=np.float32), 10, axis=1)
    pk10a[0:8, 134] = np.tile(f32("bm"), 4)
    pk10b = np.zeros((10, 2304), dtype=np.float32)
    pk10b[:, 0:256] = f32("W2")
    pk10b[:, 256:2304] = np.arange(N_STEPS, dtype=np.float32)[None, :]
    return (
        np.ascontiguousarray(pk3),
        np.ascontiguousarray(pk10a),
        np.ascontiguousarray(pk10b),
    )


def kernel(**inputs):
    nc = get_nc()
    eps = np.ascontiguousarray(
        np.asarray(inputs["eps"], dtype=np.float32).reshape(N_ARMS, F)
    )
    pk3, pk10a, pk10b = _pack_smalls(inputs)
    in_maps = [
        {
            "pk3": pk3,
            "pk10a": pk10a,
            "pk10b": pk10b,
            "eps": eps[i * ARMS_PER_CORE : (i + 1) * ARMS_PER_CORE],
        }
        for i in range(N_CORES)
    ]
    res = run_bass_kernel_spmd(nc, in_maps, core_ids=list(range(N_CORES)))
    out = np.concatenate([res.results[i]["out"] for i in range(N_CORES)], axis=0)
    return out.reshape(N_ARMS, 2, N_STEPS)


# revision 13
# speedup vs baseline: 2.1507x; 1.1737x over previous
"""Trainium2 Bass kernel for the arm-sampling rollout problem.

Math: the reference's 2048-step scan x <- x - (A@x)*dt with
A = P diag(exp(D)) P^-1 has the closed form
    hidden[k] = P diag(lam_i^k) P^-1 x0,   lam_i = 1 - dt*exp(D_i)
so actions^T[ch, k] = tanh(sum_i G[ch,i] * c_i * lam_i^k + bm[ch]) with
G = Wm @ P and c = P^-1 x0 (on-device Gauss-Jordan on [P^T | I]).
The output is the memory-bound broadcast
    out[arm, j] = 150*eps[arm, j] + 15000*act_flat[j]
over a [5000, 4096] array, 625 arms per core across 8 cores.

Key scheduling facts measured from NTFF profiles:
- A dma_start's descriptors (16KB per partition-row) are spread over
  the 16 DMA engines (26.8GB/s each) only while other transfers are
  co-resident in the queue's dispatch window; a transfer alone in the
  window crawls on ~1 engine. Each HWDGE queue also only admits ~5
  outstanding dma_starts (later triggers stall). So bulk I/O is split
  into ~1MB transfers across BOTH HWDGE queues (sync+scalar), with
  small trailing dummy transfers so no real transfer drains alone.
- PE fp32 matmul costs ~2.1us per 512 cols; bf16 ~0.55us. The action
  row is computed as ONE [8,512] fp32 matmul via the geometric-series
  split lam^(k+512j) = lam^(512j) * lam^k (lam^(512j) columns folded
  into 8 stacked lhsT columns), tanh'd to bf16, and broadcast 128-wide
  by selector-matrix bf16 matmuls (rhs = full [8,512] tile at base
  partition 0, so no cross-partition hop is needed).
- All small parameters, the identity/row-selector tables, and the
  step-index row are host-packed into three small partition-layout
  arrays (pk3, pk10a, pk10b), each loading as a handful of
  descriptors; gpsimd runs nothing but the tiny GJ column preps.
"""

import numpy as np

import concourse.bass as bass
import concourse.bacc as bacc
import concourse.mybir as mybir
import concourse.tile as tile
from concourse.bass_utils import run_bass_kernel_spmd

N_ARMS = 5000
N_STEPS = 2048
H = 10
F = 2 * N_STEPS  # 4096 flattened per-arm elements
N_CORES = 8
ARMS_PER_CORE = N_ARMS // N_CORES  # 625
FP = mybir.dt.float32
BF = mybir.dt.bfloat16

_NC_CACHE: dict = {}


def build_nc():
    AFT = mybir.ActivationFunctionType
    ALU = mybir.AluOpType

    nc = bacc.Bacc(
        "TRN2",
        target_bir_lowering=False,
        debug=False,
        enable_asserts=True,
        num_devices=N_CORES,
    )

    eps_d = nc.dram_tensor("eps", [ARMS_PER_CORE, F], FP, kind="ExternalInput")
    pk3_d = nc.dram_tensor("pk3", [3, 257], FP, kind="ExternalInput")
    pk10a_d = nc.dram_tensor("pk10a", [10, 136], FP, kind="ExternalInput")
    pk10b_d = nc.dram_tensor("pk10b", [10, 2304], FP, kind="ExternalInput")
    out_d = nc.dram_tensor("out", [ARMS_PER_CORE, F], FP, kind="ExternalOutput")
    dscr_d = nc.dram_tensor("dscr", [32, F], FP, kind="Internal")

    TILE_ROWS = [(0, 128), (128, 256), (256, 384), (384, 512), (512, 625)]
    # Trigger order matters: the per-queue credit limit (~5 outstanding)
    # staggers later triggers by completion times, and a transfer left alone
    # crawls on one DMA engine. So the small tile-4 pieces ride in the first
    # tight window (credits free early since they finish early), and the two
    # remaining big transfers + dummy form the tail set that co-finishes in
    # spread mode.
    IN_GROUPS = [
        (0, 128), (128, 256), (512, 569), (569, 625), (256, 384), (384, 512),
    ]

    with tile.TileContext(nc) as tc:
        with (
            tc.tile_pool(name="sbc", bufs=1) as sbc,
            tc.tile_pool(name="sbgj", bufs=2) as sbgj,
            tc.tile_pool(name="sbeps", bufs=1) as sbeps,
            tc.tile_pool(name="psa", bufs=3, space=bass.MemorySpace.PSUM) as psa,
            tc.tile_pool(name="psbc", bufs=2, space=bass.MemorySpace.PSUM) as psbc,
            tc.tile_pool(name="psB", bufs=2, space=bass.MemorySpace.PSUM) as psB,
        ):
            # ---------- selector table built pre-input ------------------------
            # selm[:, 128r:128(r+1)] is the [8,128] lhsT that broadcasts row r
            seli = sbc.tile([8, 8 * 128], mybir.dt.int32, tag="seli")
            nc.gpsimd.iota(
                seli[:], pattern=[[-1, 8], [0, 128]], base=0, channel_multiplier=1
            )
            selm = sbc.tile([8, 8 * 128], BF, tag="selm")
            nc.vector.tensor_scalar(selm[:], seli[:], 0, None, ALU.is_equal)

            # ---------- sync HWDGE: pk3 + eps groups 0-4 + dummy --------------
            pk3 = sbc.tile([3, 257], FP, tag="pk3")
            nc.sync.dma_start(pk3[:], pk3_d.ap())
            eps_tiles = []
            for r0, r1 in TILE_ROWS:
                t = sbeps.tile([128, F], FP, tag="eps" + str(r0))
                eps_tiles.append((t, r0, r1 - r0))

            def load_group(eng, g0, g1):
                ti = g0 // 128
                t, r0, _ = eps_tiles[ti]
                eng.dma_start(t[g0 - r0 : g1 - r0, :], eps_d.ap()[g0:g1, :])

            for g0, g1 in IN_GROUPS:
                load_group(nc.sync, g0, g1)
            # ~0.5MB dummy rides with the input tail so it drains in spread mode
            din1 = sbc.tile([32, F], FP, tag="din1")
            nc.sync.dma_start(din1[:], eps_d.ap()[0:32, :])

            # ---------- scalar HWDGE: pk10a/b (outputs come later) ------------
            pk10a = sbc.tile([10, 136], FP, tag="pk10a")
            nc.scalar.dma_start(pk10a[:], pk10a_d.ap())
            pk10b = sbc.tile([10, 2304], FP, tag="pk10b")
            nc.scalar.dma_start(pk10b[:], pk10b_d.ap())

            # pk10a: [:,0:10]=P^T [:,10:20]=P [:,20:22]=Wm^T [:,22]=D [:,23]=b2
            #        [:,24:34]=I10 [:,34:134]=oht [0:8,134]=bm8
            pT = pk10a[:, 0:10]
            p_nat = pk10a[:, 10:20]
            wmT = pk10a[:, 20:22]
            dcol = pk10a[:, 22:23]
            b2col = pk10a[:, 23:24]
            idm = pk10a[:, 24:34]
            oht = pk10a[:, 34:134]
            bm8 = pk10a[0:8, 134:135]
            # pk10b: [:,0:256]=W2, [:,256:2304]=kf (0..2047 each row)
            w2 = pk10b[:, 0:256]
            kf = pk10b[:, 256:2304]
            # pk3: rows0-1=W1^T, row2=b1; col256=[t0,t1,1]
            tgt1 = pk3[:, 256:257]

            # ---------- lam = 1 - 0.01*exp(D); vc = lam^k ---------------------
            es = sbc.tile([H, 1], FP, tag="es")
            nc.scalar.activation(es[:], dcol, AFT.Exp)
            lam = sbc.tile([H, 1], FP, tag="lam")
            nc.vector.tensor_scalar(lam[:], es[:], -0.01, 1.0, ALU.mult, ALU.add)
            lnl = sbc.tile([H, 1], FP, tag="lnl")
            nc.scalar.activation(lnl[:], lam[:], AFT.Ln)
            vc = sbc.tile([H, N_STEPS], FP, tag="vc")
            nc.scalar.activation(vc[:], kf, AFT.Exp, scale=lnl[:])

            # ---------- h = relu(W1 t + b1) via augmented-contraction mm ------
            hp0 = psa.tile([128, 1], FP, tag="mm")
            nc.tensor.matmul(hp0[:], pk3[:, 0:128], tgt1)
            hp1 = psa.tile([128, 1], FP, tag="mm")
            nc.tensor.matmul(hp1[:], pk3[:, 128:256], tgt1)
            h0 = sbc.tile([128, 1], FP, tag="h0")
            nc.scalar.activation(h0[:], hp0[:], AFT.Relu)
            h1 = sbc.tile([128, 1], FP, tag="h1")
            nc.scalar.activation(h1[:], hp1[:], AFT.Relu)

            # ---------- Gauss-Jordan on [P^T | I] -> Q = P^-T -----------------
            aug = sbgj.tile([H, 2 * H], FP, tag="aug")
            nc.vector.tensor_copy(aug[:, 0:H], pT)
            nc.vector.tensor_copy(aug[:, H : 2 * H], idm)
            for k in range(H):
                fn = sbgj.tile([H, 1], FP, tag="fn")
                nc.gpsimd.tensor_sub(fn[:], idm[:, k : k + 1], aug[:, k : k + 1])
                bc = psbc.tile([H, 2 * H], FP, tag="bc")
                nc.tensor.matmul(bc[:], oht[:, H * k : H * k + H], aug[:])
                piv = sbgj.tile([H, 1], FP, tag="piv")
                nc.vector.reciprocal(piv[:], bc[:, k : k + 1])
                S = sbgj.tile([H, 2 * H], FP, tag="S")
                nc.vector.tensor_scalar_mul(S[:], bc[:], piv[:])
                aug2 = sbgj.tile([H, 2 * H], FP, tag="aug")
                nc.vector.scalar_tensor_tensor(
                    aug2[:], S[:], fn[:], aug[:], ALU.mult, ALU.add
                )
                aug = aug2

            # ---------- W2^T, x0, c, G^T*c ------------------------------------
            w2tp0 = psa.tile([128, H], FP, tag="mm")
            nc.tensor.matmul(w2tp0[:], w2[:, 0:128], idm, is_transpose=True)
            w2tp1 = psa.tile([128, H], FP, tag="mm")
            nc.tensor.matmul(w2tp1[:], w2[:, 128:256], idm, is_transpose=True)
            w2t0 = sbc.tile([128, H], FP, tag="w2t0")
            nc.vector.tensor_copy(w2t0[:], w2tp0[:])
            w2t1 = sbc.tile([128, H], FP, tag="w2t1")
            nc.vector.tensor_copy(w2t1[:], w2tp1[:])
            x0p = psa.tile([H, 1], FP, tag="mm")
            nc.tensor.matmul(x0p[:], w2t0[:], h0[:], start=True, stop=False)
            nc.tensor.matmul(x0p[:], w2t1[:], h1[:], start=False, stop=True)
            x0s = sbc.tile([H, 1], FP, tag="x0s")
            nc.scalar.activation(x0s[:], x0p[:], AFT.Identity, bias=b2col, scale=1.0)
            gtp = psa.tile([H, 2], FP, tag="mm")
            nc.tensor.matmul(gtp[:], p_nat, wmT)
            cp = psa.tile([H, 1], FP, tag="mm")
            nc.tensor.matmul(cp[:], aug[:, H : 2 * H], x0s[:])
            gts = sbc.tile([H, 2], FP, tag="gts")
            nc.vector.tensor_scalar_mul(gts[:], gtp[:], cp[:, 0:1])

            # ---------- actions: ONE [8,512] mm via lam^(512j) folding --------
            # gstack[:, 2j+ch] = gts[:, ch] * lam^(512j); vc cols 0,512,1024,1536
            gstack = sbc.tile([H, 8], FP, tag="gstack")
            for j in range(4):
                nc.vector.tensor_scalar_mul(
                    gstack[:, 2 * j : 2 * j + 2], gts[:], vc[:, 512 * j : 512 * j + 1]
                )
            pre8 = psa.tile([8, 512], FP, tag="mm")
            nc.tensor.matmul(pre8[:], gstack[:], vc[:, 0:512])
            ats8 = sbc.tile([8, 512], BF, tag="ats8")
            nc.scalar.activation(ats8[:], pre8[:], AFT.Tanh, bias=bm8, scale=1.0)

            # ---------- B[p, 2t+ch] = 15000*row_(2j+ch)(ats8) broadcast -------
            # copies split scalar (ch0) / vector (ch1) so B's halves complete
            # as the bcast matmuls stream out of the PE
            Bsb = sbc.tile([128, F], FP, tag="B")
            B3 = Bsb[:].rearrange("p (t m) -> p t m", m=2)
            for r in range(8):
                j, ch = r // 2, r % 2
                bp = psB.tile([128, 512], FP, tag="B")
                nc.tensor.matmul(bp[:], selm[:, 128 * r : 128 * (r + 1)], ats8[:])
                dst = B3[:, 512 * j : 512 * (j + 1), ch : ch + 1]
                if ch == 0:
                    nc.scalar.activation(dst, bp[:, :, None], AFT.Copy, scale=15000.0)
                else:
                    nc.vector.tensor_scalar(
                        dst, bp[:, :, None], 15000.0, None, ALU.mult
                    )

            # ---------- main: out = 150*eps + B (half-col STTs so work can ---
            # start on B's first half); one big output DMA per tile, and the
            # last tile's output goes out as a tight burst of 2 pieces + dummy
            HW = F // 2
            for t, r0, pt in eps_tiles:
                for hh in range(2):
                    c0, c1 = hh * HW, (hh + 1) * HW
                    nc.vector.scalar_tensor_tensor(
                        t[0:pt, c0:c1],
                        t[0:pt, c0:c1],
                        150.0,
                        Bsb[0:pt, c0:c1],
                        ALU.mult,
                        ALU.add,
                    )
                if pt == 128:
                    nc.scalar.dma_start(out_d.ap()[r0 : r0 + pt, :], t[0:pt, :])
                else:
                    nc.scalar.dma_start(out_d.ap()[r0 : r0 + 57, :], t[0:57, :])
                    nc.scalar.dma_start(out_d.ap()[r0 + 57 : r0 + pt, :], t[57:pt, :])
                    nc.scalar.dma_start(dscr_d.ap(), Bsb[0:32, :])

    nc.compile()
    return nc


def get_nc():
    if "nc" not in _NC_CACHE:
        _NC_CACHE["nc"] = build_nc()
    return _NC_CACHE["nc"]


def _pack_smalls(inputs):
    f32 = lambda k: np.asarray(inputs[k], dtype=np.float32)
    pk3 = np.zeros((3, 257), dtype=np.float32)
    pk3[0:2, 0:256] = f32("W1").T
    pk3[2, 0:256] = f32("b1")
    pk3[0:2, 256] = f32("target")
    pk3[2, 256] = 1.0
    pk10a = np.zeros((10, 136), dtype=np.float32)
    pk10a[:, 0:10] = f32("P").T
    pk10a[:, 10:20] = f32("P")
    pk10a[:, 20:22] = f32("Wm").T
    pk10a[:, 22] = f32("D")
    pk10a[:, 23] = f32("b2")
    pk10a[:, 24:34] = np.eye(10, dtype=np.float32)
    pk10a[:, 34:134] = np.repeat(np.eye(10, dtype=np.float32), 10, axis=1)
    pk10a[0:8, 134] = np.tile(f32("bm"), 4)
    pk10b = np.zeros((10, 2304), dtype=np.float32)
    pk10b[:, 0:256] = f32("W2")
    pk10b[:, 256:2304] = np.arange(N_STEPS, dtype=np.float32)[None, :]
    return (
        np.ascontiguousarray(pk3),
        np.ascontiguousarray(pk10a),
        np.ascontiguousarray(pk10b),
    )


def kernel(**inputs):
    nc = get_nc()
    eps = np.ascontiguousarray(
        np.asarray(inputs["eps"], dtype=np.float32).reshape(N_ARMS, F)
    )
    pk3, pk10a, pk10b = _pack_smalls(inputs)
    in_maps = [
        {
            "pk3": pk3,
            "pk10a": pk10a,
            "pk10b": pk10b,
            "eps": eps[i * ARMS_PER_CORE : (i + 1) * ARMS_PER_CORE],
        }
        for i in range(N_CORES)
    ]
    res = run_bass_kernel_spmd(nc, in_maps, core_ids=list(range(N_CORES)))
    out = np.concatenate([res.results[i]["out"] for i in range(N_CORES)], axis=0)
    return out.reshape(N_ARMS, 2, N_STEPS)
